# revision 16
# baseline (speedup 1.0000x reference)
"""Trainium2 Bass kernel for AgnosticChargeBiasedLinearPotentialEmbedding.

Math (per node n, for each irrep block l in {0,1,2} with multiplicity 128 and
m in 0..2l):
    out[n, off_l + o*(2l+1) + m] =
        (1/sqrt(128)) * sum_i node_feats[n, off_l + i*(2l+1) + m] * Wn_l[i, o]
        + potential_feats[n, poff_l + m] * Wp_l[0, o]
        + (l == 0) * local_charges[n, 0] * Wc0[0, o]

Device strategy (data-parallel over nodes, 8 cores):
  - Host pre-transposes/deinterleaves node_feats into XT[i, lm, n] (fp16) so
    the TensorE can stream node-columns against a stationary W_l (3 weight
    loads per superblock, zero on-chip transposes).
  - Main matmul per (lm, 512-node chunk): out.T[o, n] = W_l[i,o].T-contract
    X.T[i, n] into PSUM.
  - Rank-1 (potential/charge) terms are folded in as a second accumulating
    K=10 matmul per chunk from a tiny [10, n] host tensor.
  - PSUM drained to SBUF alternately by VectorE / ScalarE (plain copies),
    stored transposed; host reassembles the natural [N, 1152] layout.
"""

import math

import numpy as np

import concourse.bass as bass
import concourse.tile as tile
from concourse import bacc, mybir
from concourse.bass_utils import run_bass_kernel_spmd

# Problem constants (hardcoded per contract; kernel.py must be self-contained).
N = 100000
N_CORES = 8
N_PER_CORE = 12800          # padded so 8 * 12800 = 102400 >= 100000
N_PAD = N_CORES * N_PER_CORE
LMS = [(0, 0), (1, 0), (1, 1), (1, 2), (2, 0), (2, 1), (2, 2), (2, 3), (2, 4)]
L_OFF = {0: 0, 1: 128, 2: 512}      # node-feats column offset of each l block
P_OFF = {0: 0, 1: 1, 2: 4}          # potential-feats column offset of each l
CHUNK = 512                  # matmul moving free dim (one PSUM bank of fp32)
SUPER = 2048                 # nodes per superblock (4 chunks -> 4 PSUM banks)
K_RANK1 = 128                # 9 potential rows + 1 charge row, zero-padded
                             # to full K so HAM sees full-array activity

IN_DT = mybir.dt.float16
OUT_DT = mybir.dt.float16
IN_NP = np.float16
OUT_NP = np.float16


def _build_bass():
    nc = bacc.Bacc("TRN2", num_devices=N_CORES)

    # xt is packed superblock-major on the host: for each superblock the
    # [9, sb] block of every partition row is contiguous, so a superblock
    # load is 128 descriptors of 9*sb*2 contiguous bytes.
    xt = nc.declare_dram_parameter("xt", [128, 9 * N_PER_CORE], IN_DT, isOutput=False)
    w = nc.declare_dram_parameter("w", [128, 3, 128], IN_DT, isOutput=False)
    rw = nc.declare_dram_parameter("rw", [K_RANK1, 9, 128], IN_DT, isOutput=False)
    # Only the leading rows carry data (9 potential + 1 charge + pad to the
    # 32-partition alignment); rows 32..127 of the SBUF tile are zeroed once
    # on-chip instead of shipping ~2.5 MB of zeros per core over HBM.
    pt = nc.declare_dram_parameter("pt", [32, N_PER_CORE], IN_DT, isOutput=False)
    out_t = nc.declare_dram_parameter("out_t", [9, 128, N_PER_CORE], OUT_DT, isOutput=True)

    # superblock sizes covering N_PER_CORE
    supers = []
    pos = 0
    while pos < N_PER_CORE:
        sb = min(SUPER, N_PER_CORE - pos)
        supers.append((pos, sb))
        pos += sb

    with tile.TileContext(nc) as tc:
        with (
            tc.tile_pool(name="const", bufs=1) as const_pool,
            tc.tile_pool(name="xw", bufs=3) as x_pool,
            tc.tile_pool(name="psum", bufs=2, space=bass.MemorySpace.PSUM) as psum_pool,
            tc.tile_pool(name="osb", bufs=6) as o_pool,
        ):
            # Resident constants (scalar/gpsimd rings so the first X-superblock
            # load on the sync ring starts immediately).
            pt_sb = const_pool.tile([K_RANK1, N_PER_CORE], IN_DT, tag="pt")
            for pb in (32, 64, 96):
                nc.gpsimd.memset(pt_sb[pb:pb + 32, :], 0.0)
            nc.scalar.dma_start(pt_sb[0:32, :], pt[:])
            w_sb = const_pool.tile([128, 3, 128], IN_DT, tag="w")
            nc.scalar.dma_start(w_sb[:], w[:])
            rw_sb = const_pool.tile([K_RANK1, 9, 128], IN_DT, tag="rw")
            nc.scalar.dma_start(rw_sb[:], rw[:])

            drain_idx = 0
            for pos, sb in supers:
                xw = x_pool.tile([128, 9, sb], IN_DT, tag="xw")
                nc.sync.dma_start(
                    xw[:], xt[:, 9 * pos:9 * (pos + sb)].rearrange(
                        "p (g n) -> p g n", g=9)
                )

                for lm, (l, _m) in enumerate(LMS):
                    ps = psum_pool.tile([128, sb], mybir.dt.float32, tag="ps")
                    # rank-1 (potential/charge) pass first: one LDW, sb/CHUNK
                    # matmuls, then the main pass: one LDW, sb/CHUNK matmuls.
                    # Grouping by stationary operand keeps LDW hidden.
                    for c0 in range(0, sb, CHUNK):
                        c1 = min(c0 + CHUNK, sb)
                        nc.tensor.matmul(
                            ps[:, c0:c1],
                            rw_sb[:, lm, :],
                            pt_sb[:, pos + c0:pos + c1],
                            start=True,
                            stop=False,
                        )
                    for c0 in range(0, sb, CHUNK):
                        c1 = min(c0 + CHUNK, sb)
                        nc.tensor.matmul(
                            ps[:, c0:c1],
                            w_sb[:, l, :],
                            xw[:, lm, c0:c1],
                            start=False,
                            stop=True,
                        )
                    osb = o_pool.tile([128, sb], OUT_DT, tag="osb")
                    if drain_idx % 2 == 0:
                        nc.vector.tensor_copy(osb[:], ps[:])
                    else:
                        nc.scalar.copy(osb[:], ps[:])
                    store_eng = nc.scalar if drain_idx % 2 == 0 else nc.gpsimd
                    drain_idx += 1
                    store_eng.dma_start(out_t[lm, :, pos:pos + sb], osb[:])

    nc.compile()
    return nc


def _host_pack(potential_feats, node_feats, local_charges):
    """Build the device-side input tensors (all fp16)."""
    inv = 1.0 / math.sqrt(128.0)

    # XT[i, lm, n]: deinterleaved transpose of node_feats.
    xt = np.zeros((128, 9, N_PAD), dtype=IN_NP)
    for lm, (l, m) in enumerate(LMS):
        d = 2 * l + 1
        blk = node_feats[:, L_OFF[l] + m:L_OFF[l] + 128 * d:d]   # [N, 128]
        xt[:, lm, :N] = blk.T.astype(IN_NP)
    # Repack superblock-major per core: per partition row, each superblock's
    # [9, sb] block contiguous -> [128, 9*N_PER_CORE] per core.
    xt_sb = np.empty((128, N_CORES, 9 * N_PER_CORE), dtype=IN_NP)
    for c in range(N_CORES):
        base = c * N_PER_CORE
        pos = 0
        while pos < N_PER_CORE:
            sb = min(SUPER, N_PER_CORE - pos)
            xt_sb[:, c, 9 * pos:9 * (pos + sb)] = (
                xt[:, :, base + pos:base + pos + sb].reshape(128, 9 * sb))
            pos += sb
    xt = xt_sb

    # PT[k, n]: 9 potential rows (lm order) + charge row (+ zero pad to 32).
    ptm = np.zeros((32, N_PAD), dtype=IN_NP)
    for lm, (l, m) in enumerate(LMS):
        ptm[lm, :N] = potential_feats[:, P_OFF[l] + m].astype(IN_NP)
    ptm[9, :N] = local_charges[:, 0].astype(IN_NP)
    return xt, ptm, inv


def _host_weights(Wp0, Wp1, Wp2, Wn0, Wn1, Wn2, Wc0):
    inv = 1.0 / math.sqrt(128.0)
    w = np.stack([Wn0 * inv, Wn1 * inv, Wn2 * inv], axis=1).astype(IN_NP)  # [128,3,128]
    rw = np.zeros((K_RANK1, 9, 128), dtype=IN_NP)
    wp = {0: Wp0, 1: Wp1, 2: Wp2}
    for lm, (l, _m) in enumerate(LMS):
        rw[lm, lm, :] = wp[l][0].astype(IN_NP)
    rw[9, 0, :] = Wc0[0].astype(IN_NP)
    return w, rw


def _host_unpack(outs):
    """outs: list of 8 [9, 128, N_PER_CORE] arrays -> [N, 1152] fp32."""
    full = np.concatenate(outs, axis=2)        # [9, 128, N_PAD]
    out = np.empty((N, 1152), dtype=np.float32)
    lm = 0
    for l in (0, 1, 2):
        d = 2 * l + 1
        # rows lm..lm+d-1 -> [d, 128, N] -> natural [N, 128, d]
        blk = full[lm:lm + d, :, :N].astype(np.float32)
        out[:, L_OFF[l]:L_OFF[l] + 128 * d] = blk.transpose(2, 1, 0).reshape(N, 128 * d)
        lm += d
    return out


_NC_CACHE = {}


def _get_nc():
    if "nc" not in _NC_CACHE:
        _NC_CACHE["nc"] = _build_bass()
    return _NC_CACHE["nc"]


def _build_in_maps(potential_feats, node_feats, local_charges,
                   Wp0, Wp1, Wp2, Wn0, Wn1, Wn2, Wc0):
    xt, ptm, _ = _host_pack(potential_feats, node_feats, local_charges)
    w, rw = _host_weights(Wp0, Wp1, Wp2, Wn0, Wn1, Wn2, Wc0)
    in_maps = []
    for c in range(N_CORES):
        s = slice(c * N_PER_CORE, (c + 1) * N_PER_CORE)
        in_maps.append({
            "xt": np.ascontiguousarray(xt[:, c, :]),
            "w": w,
            "rw": rw,
            "pt": np.ascontiguousarray(ptm[:, s]),
        })
    return in_maps


def kernel(potential_feats, node_feats, node_attrs, local_charges,
           Wp0, Wp1, Wp2, Wn0, Wn1, Wn2, Wc0):
    del node_attrs  # explicitly unused in the reference forward
    in_maps = _build_in_maps(
        np.asarray(potential_feats, np.float32),
        np.asarray(node_feats, np.float32),
        np.asarray(local_charges, np.float32),
        np.asarray(Wp0, np.float32), np.asarray(Wp1, np.float32),
        np.asarray(Wp2, np.float32), np.asarray(Wn0, np.float32),
        np.asarray(Wn1, np.float32), np.asarray(Wn2, np.float32),
        np.asarray(Wc0, np.float32),
    )
    nc = _get_nc()
    res = run_bass_kernel_spmd(nc, in_maps, list(range(N_CORES)))
    outs = [res.results[c]["out_t"] for c in range(N_CORES)]
    return _host_unpack(outs)
